# revision 24
# baseline (speedup 1.0000x reference)
"""Trainium2 Bass kernel for a 4-layer GCN (N=50000, D=128, E=1600000, 8 cores).

Hybrid aggregation (v3, group-major pipeline):
  - Nodes padded to 50176 = 392*128; each core owns 6272 nodes (49 tiles).
  - out = dis * ((sum_{e->v} x'[src_e]) @ W) + b + prev, x' = dis * x.
  - Src tiles split: DENSE = global tiles [0, M_DENSE) (same for all cores so
    the SPMD stream is identical; per-core B data differs), GATHER = the rest.
    Self-loops via an identity matmul from the SBUF-resident x' own slice.
  - 49 dst tiles -> 13 groups of <=4 (512 dst columns, one PSUM bank each),
    processed as a pipeline; per group:
      dense:  psum[fi,dst] += xall[:,s,:]^T @ B[s, g-slice]   (fp8 B streamed
              from HBM in 24-src-tile sub-blocks, N=512 moving operands)
      gather: dma_gather x' rows (bf16 edge-major, ~8ns/row Q7) + one-hot fp8
              S chunks: psum += msg_chunk^T @ S_chunk  (per 2-tile units)
      self:   psum[:, tl] += xpr_own_tile^T @ I
      close:  aggT bf16 <- psum; per tile: @W -> psum_out; epilogue on DVE/ACT
  - x' slices AllGather'd into the next layer's table; dense x' tiles bulk
    reloaded to SBUF each layer (no per-edge descriptors on the dense path).
"""

import sys

sys.path.insert(0, "/opt/trn_rl_repo")

import numpy as np
import ml_dtypes

N = 50000
D = 128
L = 4
E = 1600000
NCORES = 8
NPAD = 50176          # 392 * 128
NT = NPAD // 128      # 392 src tiles
NPC = NPAD // NCORES  # 6272
TPC = NPC // 128      # 49
M_DENSE = 192         # dense src tiles (global ids [0, M_DENSE))
DENSE_ROWS = M_DENSE * 128  # 24576; gathered span 25600 < 2^15
SBW = 8               # dense src tiles per B sub-block DMA
NSB = M_DENSE // SBW  # 24
GROUP = 4
NGROUPS = (TPC + GROUP - 1) // GROUP  # 13 (last group = 1 tile)
GW = 512              # psum group width (4*128)

_compiled = None


def _units_of_group(tg):
    return [tg[i:i + 2] for i in range(0, len(tg), 2)]


def _node_perm():
    """old node id -> rank-major three-zone table row.

    Zone A1 (rows [0, 12288)): core c's tiles 0..11 at c*1536 + t*128 + p.
    Zone A2 (rows [12288, 24576)): tiles 12..23 at 12288 + c*1536 + (t-12)*128 + p.
    Zone B (rows [24576, 50176)): tiles 24..48 at 24576 + c*3200 + (t-24)*128 + p.
    """
    n = np.arange(NPAD, dtype=np.int64)
    c = n // NPC
    r = n % NPC
    t = r // 128
    p = r % 128
    return np.where(
        t < 12, c * 1536 + t * 128 + p,
        np.where(t < 24, 12288 + c * 1536 + (t - 12) * 128 + p,
                 DENSE_ROWS + c * 3200 + (t - 24) * 128 + p))


def _preprocess(x, edge_index, W, b):
    fp8 = ml_dtypes.float8_e4m3
    src_old = edge_index[0].astype(np.int64)
    dst0 = edge_index[1].astype(np.int64)
    loops = np.arange(N, dtype=np.int64)
    deg = np.bincount(np.concatenate([dst0, loops]), minlength=N).astype(np.float32)
    dis = np.zeros(NPAD, np.float32)
    dis[:N] = 1.0 / np.sqrt(deg)

    perm = _node_perm()
    src0 = perm[src_old]
    core_of = dst0 // NPC
    is_dense = src0 < DENSE_ROWS

    # ---------------- dense path: B count blocks, group-major ----------------
    # BG[c] layout: [NGROUPS, NSB, 128(u), SBW(k), GW(dst col)]
    BG = np.zeros((NCORES, NGROUPS, NSB, 128, SBW, GW), np.uint8)
    de = np.where(is_dense)[0]
    d_core = core_of[de]
    st = src0[de] // 128
    d_sb = st // SBW
    d_k = st % SBW
    d_u = src0[de] % 128
    d_dloc = dst0[de] - d_core * NPC
    d_g = d_dloc // GW
    d_col = d_dloc % GW
    np.add.at(BG, (d_core, d_g, d_sb, d_u, d_k, d_col), 1)

    # ---------------- gather path ----------------
    ge = np.where(~is_dense)[0]
    gsrc, gdst = src0[ge], dst0[ge]
    g_core = core_of[ge]
    lt_of = (gdst % NPC) // 128
    dloc_of = gdst % 128
    key = g_core * TPC + lt_of
    order = np.argsort(key, kind="stable")
    src_s = gsrc[order]
    dloc_s = dloc_of[order]
    counts = np.bincount(key, minlength=NCORES * TPC).reshape(NCORES, TPC)
    KCH = np.maximum(np.ceil(counts.max(axis=0) / 128.0).astype(np.int64), 1)  # [TPC]

    tiles_of_group = [list(range(g * GROUP, min((g + 1) * GROUP, TPC)))
                      for g in range(NGROUPS)]
    TOTCH = int(KCH.sum())
    chunk_off = np.zeros(TPC, np.int64)
    pos = 0
    for tg in tiles_of_group:
        for t in tg:
            chunk_off[t] = pos
            pos += KCH[t]
    assert pos == TOTCH

    idx_all = np.zeros((NCORES, TOTCH * 128), np.int16)
    dlocs = np.full((NCORES, TOTCH * 128), -1, np.int16)
    starts = np.zeros(NCORES * TPC + 1, np.int64)
    np.cumsum(counts.reshape(-1), out=starts[1:])
    for c in range(NCORES):
        for t in range(TPC):
            k = c * TPC + t
            s0, s1 = starts[k], starts[k + 1]
            n = s1 - s0
            o = chunk_off[t] * 128
            idx_all[c, o:o + n] = (src_s[s0:s1] - DENSE_ROWS).astype(np.int16)
            dlocs[c, o:o + n] = dloc_s[s0:s1]

    smat = np.zeros((NCORES, TOTCH, 128, 128), fp8)
    ii = np.arange(TOTCH * 128)
    for c in range(NCORES):
        d = dlocs[c]
        m = d >= 0
        smat[c][ii[m] // 128, ii[m] % 128, d[m]] = 1.0

    idxw = np.zeros((NCORES, 128, TOTCH * 8), np.int16)
    for c in range(NCORES):
        w = idx_all[c].reshape(TOTCH * 8, 16).T
        for gme in range(8):
            idxw[c, gme * 16:(gme + 1) * 16, :] = w

    xpad = np.zeros((NPAD, D), np.float32)
    xpad[:N] = x
    x0p_node = (xpad * dis[:, None]).astype(ml_dtypes.bfloat16)
    x0p = np.empty_like(x0p_node)
    x0p[perm] = x0p_node  # table in rank-major-by-half layout
    disT = dis.reshape(NCORES, TPC, 128).transpose(0, 2, 1).copy()  # [c,128,TPC]
    b_bc = np.broadcast_to(b[None, :, :], (128, L, D)).astype(np.float32).copy()
    ident = np.eye(128, dtype=np.float32).astype(ml_dtypes.bfloat16)

    meta = dict(KCH=KCH, TOTCH=TOTCH, chunk_off=chunk_off,
                tiles_of_group=tiles_of_group)
    per_core = []
    for c in range(NCORES):
        x0p_own = np.ascontiguousarray(
            x0p_node[c * NPC:(c + 1) * NPC].reshape(TPC, 128, D).transpose(1, 0, 2))
        per_core.append(dict(
            x_own=np.ascontiguousarray(xpad[c * NPC:(c + 1) * NPC]),
            x0p=x0p,
            x0p_own=x0p_own,
            w=W.astype(ml_dtypes.bfloat16),
            b_bc=b_bc,
            disT=np.ascontiguousarray(disT[c]),
            ident=ident,
            idxs=np.ascontiguousarray(idxw[c]),
            smat=np.ascontiguousarray(smat[c].transpose(1, 0, 2)),
            bg=BG[c].astype(fp8),
        ))
    return meta, per_core


def _build(meta):
    from concourse import bacc, tile
    from concourse.bass import mybir

    KCH = meta["KCH"]
    TOTCH = meta["TOTCH"]
    chunk_off = meta["chunk_off"]
    tiles_of_group = meta["tiles_of_group"]
    MAXCH_U = max(int(sum(KCH[t] for t in u))
                  for tg in tiles_of_group for u in _units_of_group(tg))

    nc = bacc.Bacc("TRN2", target_bir_lowering=False, debug=False,
                   num_devices=NCORES)
    d_x_own = nc.dram_tensor("x_own", [NPC, D], mybir.dt.float32, kind="ExternalInput")
    d_x0p = nc.dram_tensor("x0p", [NPAD, D], mybir.dt.bfloat16, kind="ExternalInput")
    d_x0po = nc.dram_tensor("x0p_own", [128, TPC, D], mybir.dt.bfloat16, kind="ExternalInput")
    d_w = nc.dram_tensor("w", [L, D, D], mybir.dt.bfloat16, kind="ExternalInput")
    d_bbc = nc.dram_tensor("b_bc", [128, L, D], mybir.dt.float32, kind="ExternalInput")
    d_disT = nc.dram_tensor("disT", [128, TPC], mybir.dt.float32, kind="ExternalInput")
    d_ident = nc.dram_tensor("ident", [128, 128], mybir.dt.bfloat16, kind="ExternalInput")
    d_idxs = nc.dram_tensor("idxs", [128, TOTCH * 8], mybir.dt.int16, kind="ExternalInput")
    d_smat = nc.dram_tensor("smat", [128, TOTCH, 128], mybir.dt.float8e4, kind="ExternalInput")
    d_bg = nc.dram_tensor("bg", [NGROUPS, NSB, 128, SBW, GW], mybir.dt.float8e4,
                          kind="ExternalInput")
    d_out = nc.dram_tensor("x_out", [NPC, D], mybir.dt.float32, kind="ExternalOutput")

    with tile.TileContext(nc) as tc:
        with (
            tc.tile_pool(name="const", bufs=1) as constp,
            tc.tile_pool(name="xallp", bufs=1) as xallp,
            tc.tile_pool(name="msg", bufs=5) as msgp,
            tc.tile_pool(name="sp", bufs=5) as sp_pool,
            tc.tile_pool(name="bblk", bufs=2) as bbp,
            tc.tile_pool(name="work", bufs=3) as workp,
            tc.tile_pool(name="aggt", bufs=2) as aggp,
            tc.tile_pool(name="pag", bufs=4, space="PSUM") as pagp,
            tc.tile_pool(name="pout", bufs=2, space="PSUM") as poutp,
            tc.tile_pool(name="dram", bufs=1, space="DRAM") as dramp,
        ):
            x_own = constp.tile([128, TPC, D], mybir.dt.float32, tag="x_own")
            nc.sync.dma_start(out=x_own[:], in_=d_x_own.ap().rearrange("(t p) f -> p t f", p=128))
            xpr_all = constp.tile([128, TPC, D], mybir.dt.bfloat16, tag="xpr_all")
            nc.sync.dma_start(out=xpr_all[:], in_=d_x0po.ap())
            w_sb = constp.tile([128, L, D], mybir.dt.bfloat16, tag="w_sb")
            nc.sync.dma_start(out=w_sb[:], in_=d_w.ap().rearrange("l k f -> k l f"))
            bbc_sb = constp.tile([128, L, D], mybir.dt.float32, tag="bbc")
            nc.sync.dma_start(out=bbc_sb[:], in_=d_bbc.ap())
            disT_sb = constp.tile([128, TPC], mybir.dt.float32, tag="disT")
            nc.sync.dma_start(out=disT_sb[:], in_=d_disT.ap())
            ident_sb = constp.tile([128, 128], mybir.dt.bfloat16, tag="ident")
            nc.sync.dma_start(out=ident_sb[:], in_=d_ident.ap())
            idx_sb = constp.tile([128, TOTCH * 8], mybir.dt.int16, tag="idx")
            nc.sync.dma_start(out=idx_sb[:], in_=d_idxs.ap())

            tables = [d_x0p.ap()]
            ag_ins_a1 = []
            ag_ins_a2 = []
            ag_ins_b = []
            for l in range(1, L):
                tab_tile = dramp.tile([NPAD, D], mybir.dt.bfloat16, tag=f"tab{l}")
                agina1_tile = dramp.tile([12 * 128, D], mybir.dt.bfloat16, tag=f"agina1{l}")
                agina2_tile = dramp.tile([12 * 128, D], mybir.dt.bfloat16, tag=f"agina2{l}")
                aginb_tile = dramp.tile([25 * 128, D], mybir.dt.bfloat16, tag=f"aginb{l}")
                tables.append(tab_tile[:])
                ag_ins_a1.append(agina1_tile[:])
                ag_ins_a2.append(agina2_tile[:])
                ag_ins_b.append(aginb_tile[:])

            GROUP_ORDER = list(range(6, NGROUPS)) + list(range(0, 6))
            for l in range(L):
                table = tables[l]
                tabG = table[DENSE_ROWS:NPAD, :]
                xall = xallp.tile([128, M_DENSE, D], mybir.dt.bfloat16,
                                  tag="xall", name=f"xall_{l}")
                nc.sync.dma_start(
                    out=xall[:, 0:96, :],
                    in_=table[0:12288, :].rearrange("(t p) f -> p t f", p=128))
                nc.sync.dma_start(
                    out=xall[:, 96:192, :],
                    in_=table[12288:DENSE_ROWS, :].rearrange("(t p) f -> p t f", p=128))
                for g in GROUP_ORDER:
                    tg = tiles_of_group[g]
                    psum_g = pagp.tile([128, GW], mybir.dt.float32, tag="pag",
                                       name=f"pag_{l}_{g}")
                    units = []
                    for u in _units_of_group(tg):
                        nch = int(sum(KCH[t] for t in u))
                        goff = int(chunk_off[u[0]])
                        s_t = sp_pool.tile([128, MAXCH_U, 128], mybir.dt.float8e4,
                                           tag="s_t", name=f"s_t_{l}_{g}_{u[0]}")
                        nc.sync.dma_start(
                            out=s_t[:, 0:nch, :],
                            in_=d_smat.ap()[:, goff:goff + nch, :])
                        msg = msgp.tile([128, MAXCH_U, D], mybir.dt.bfloat16,
                                        tag="msg", name=f"msg_{l}_{g}_{u[0]}")
                        nc.gpsimd.dma_gather(
                            out_ap=msg[:, 0:nch, :],
                            in_ap=tabG,
                            idxs_ap=idx_sb[:, goff * 8:(goff + nch) * 8],
                            num_idxs=nch * 128,
                            num_idxs_reg=nch * 128,
                            elem_size=D,
                            single_packet=False,
                        )
                        units.append((u, goff, s_t, msg))
                    for sb in range(NSB):
                        bsub = bbp.tile([128, SBW, GW], mybir.dt.float8e4, tag="bsub",
                                        name=f"bsub_{l}_{g}_{sb}")
                        nc.sync.dma_start(out=bsub[:], in_=d_bg.ap()[g, sb])
                        for k in range(SBW):
                            nc.tensor.matmul(
                                psum_g[:],
                                lhsT=xall[:, sb * SBW + k, :],
                                rhs=bsub[:, k, :],
                                start=(sb == 0 and k == 0), stop=False)
                    for u, goff, s_t, msg in units:
                        for t in u:
                            tl = t - tg[0]
                            oT = int(chunk_off[t] - goff)
                            for c in range(int(KCH[t])):
                                nc.tensor.matmul(
                                    psum_g[:, tl * 128:(tl + 1) * 128],
                                    lhsT=msg[:, oT + c, :],
                                    rhs=s_t[:, oT + c, :],
                                    start=False, stop=False)
                    for tl, t in enumerate(tg):
                        nc.tensor.matmul(
                            psum_g[:, tl * 128:(tl + 1) * 128],
                            lhsT=xpr_all[:, t, :],
                            rhs=ident_sb[:],
                            start=False, stop=(tl == len(tg) - 1))
                    wid = len(tg) * 128
                    aggT = aggp.tile([128, GW], mybir.dt.bfloat16, tag="aggT",
                                     name=f"aggT_{l}_{g}")
                    nc.scalar.copy(aggT[:, 0:wid], psum_g[:, 0:wid])
                    for tl, t in enumerate(tg):
                        pso = poutp.tile([128, 128], mybir.dt.float32, tag="pout",
                                         name=f"pout_{l}_{t}")
                        nc.tensor.matmul(pso[:], lhsT=aggT[:, tl * 128:(tl + 1) * 128],
                                         rhs=w_sb[:, l, :], start=True, stop=True)
                        prevb = workp.tile([128, 128], mybir.dt.float32, tag="prevb")
                        nc.vector.tensor_tensor(
                            out=prevb[:], in0=x_own[:, t, :], in1=bbc_sb[:, l, :],
                            op=mybir.AluOpType.add)
                        t2 = workp.tile([128, 128], mybir.dt.float32, tag="t2")
                        nc.vector.tensor_scalar(
                            out=t2[:], in0=pso[:], scalar1=disT_sb[:, t:t + 1],
                            scalar2=None, op0=mybir.AluOpType.mult)
                        t3 = workp.tile([128, 128], mybir.dt.float32, tag="t3")
                        nc.vector.tensor_tensor(
                            out=t3[:], in0=t2[:], in1=prevb[:], op=mybir.AluOpType.add)
                        nc.scalar.activation(
                            out=x_own[:, t, :], in_=t3[:],
                            func=mybir.ActivationFunctionType.Relu)
                        if l < L - 1:
                            nc.scalar.activation(
                                out=xpr_all[:, t, :], in_=x_own[:, t, :],
                                func=mybir.ActivationFunctionType.Copy,
                                scale=disT_sb[:, t:t + 1])
                    if l < L - 1:
                        t0, t1 = tg[0], tg[-1] + 1
                        if t0 >= 24:
                            nc.sync.dma_start(
                                out=ag_ins_b[l].rearrange("(t p) f -> p t f", p=128)
                                    [:, t0 - 24:t1 - 24, :],
                                in_=xpr_all[:, t0:t1, :])
                        elif t0 >= 12:
                            nc.sync.dma_start(
                                out=ag_ins_a2[l].rearrange("(t p) f -> p t f", p=128)
                                    [:, t0 - 12:t1 - 12, :],
                                in_=xpr_all[:, t0:t1, :])
                        else:
                            nc.sync.dma_start(
                                out=ag_ins_a1[l].rearrange("(t p) f -> p t f", p=128)
                                    [:, t0:t1, :],
                                in_=xpr_all[:, t0:t1, :])
                    # progressive AllGathers: B after the part-B groups
                    # (processed first) so next layer's gathers never wait;
                    # A1 after groups 0-2 so the dense xall first half is
                    # ready before the next layer starts; A2 at layer end.
                    if l < L - 1 and g == NGROUPS - 1:
                        nc.gpsimd.collective_compute(
                            "AllGather",
                            mybir.AluOpType.bypass,
                            replica_groups=[list(range(NCORES))],
                            ins=[ag_ins_b[l].opt()],
                            outs=[tables[l + 1][DENSE_ROWS:NPAD, :].opt()],
                        )
                    if l < L - 1 and g == 2:
                        nc.gpsimd.collective_compute(
                            "AllGather",
                            mybir.AluOpType.bypass,
                            replica_groups=[list(range(NCORES))],
                            ins=[ag_ins_a1[l].opt()],
                            outs=[tables[l + 1][0:12288, :].opt()],
                        )
                    if l < L - 1 and g == 5:
                        nc.gpsimd.collective_compute(
                            "AllGather",
                            mybir.AluOpType.bypass,
                            replica_groups=[list(range(NCORES))],
                            ins=[ag_ins_a2[l].opt()],
                            outs=[tables[l + 1][12288:DENSE_ROWS, :].opt()],
                        )
            nc.sync.dma_start(out=d_out.ap().rearrange("(t p) f -> p t f", p=128),
                              in_=x_own[:])

    nc.compile()
    return nc


def kernel(x, edge_index, W, b):
    global _compiled
    from concourse import bass_utils

    x = np.asarray(x, dtype=np.float32)
    W_np = np.asarray(W, dtype=np.float32)
    b_np = np.asarray(b, dtype=np.float32)
    ei = np.asarray(edge_index)

    meta, per_core = _preprocess(x, ei, W_np, b_np)
    globals()["_last_per_core"] = per_core
    if _compiled is None:
        _compiled = _build(meta)
    nc = _compiled
    res = bass_utils.run_bass_kernel_spmd(nc, per_core, core_ids=list(range(NCORES)))
    out = np.concatenate([res.results[c]["x_out"] for c in range(NCORES)], axis=0)
    return out[:N].astype(np.float32)


# revision 25
# speedup vs baseline: 1.1266x; 1.1266x over previous
"""Trainium2 Bass kernel for a 4-layer GCN (N=50000, D=128, E=1600000, 8 cores).

Hybrid aggregation (v3, group-major pipeline):
  - Nodes padded to 50176 = 392*128; each core owns 6272 nodes (49 tiles).
  - out = dis * ((sum_{e->v} x'[src_e]) @ W) + b + prev, x' = dis * x.
  - Src tiles split: DENSE = global tiles [0, M_DENSE) (same for all cores so
    the SPMD stream is identical; per-core B data differs), GATHER = the rest.
    Self-loops via an identity matmul from the SBUF-resident x' own slice.
  - 49 dst tiles -> 13 groups of <=4 (512 dst columns, one PSUM bank each),
    processed as a pipeline; per group:
      dense:  psum[fi,dst] += xall[:,s,:]^T @ B[s, g-slice]   (fp8 B streamed
              from HBM in 24-src-tile sub-blocks, N=512 moving operands)
      gather: dma_gather x' rows (bf16 edge-major, ~8ns/row Q7) + one-hot fp8
              S chunks: psum += msg_chunk^T @ S_chunk  (per 2-tile units)
      self:   psum[:, tl] += xpr_own_tile^T @ I
      close:  aggT bf16 <- psum; per tile: @W -> psum_out; epilogue on DVE/ACT
  - x' slices AllGather'd into the next layer's table; dense x' tiles bulk
    reloaded to SBUF each layer (no per-edge descriptors on the dense path).
"""

import sys

sys.path.insert(0, "/opt/trn_rl_repo")

import numpy as np
import ml_dtypes

N = 50000
D = 128
L = 4
E = 1600000
NCORES = 8
NPAD = 50176          # 392 * 128
NT = NPAD // 128      # 392 src tiles
NPC = NPAD // NCORES  # 6272
TPC = NPC // 128      # 49
M_DENSE = 192         # dense src tiles (global ids [0, M_DENSE))
DENSE_ROWS = M_DENSE * 128  # 24576; gathered span 25600 < 2^15
SBW = 16              # dense src tiles per B sub-block DMA
NSB = M_DENSE // SBW  # 12
GROUP = 4
NGROUPS = (TPC + GROUP - 1) // GROUP  # 13 (last group = 1 tile)
GW = 512              # psum group width (4*128)

_compiled = None


def _units_of_group(tg):
    return [tg[i:i + 2] for i in range(0, len(tg), 2)]


def _node_perm():
    """old node id -> rank-major three-zone table row.

    Zone A1 (rows [0, 12288)): core c's tiles 0..11 at c*1536 + t*128 + p.
    Zone A2 (rows [12288, 24576)): tiles 12..23 at 12288 + c*1536 + (t-12)*128 + p.
    Zone B (rows [24576, 50176)): tiles 24..48 at 24576 + c*3200 + (t-24)*128 + p.
    """
    n = np.arange(NPAD, dtype=np.int64)
    c = n // NPC
    r = n % NPC
    t = r // 128
    p = r % 128
    return np.where(
        t < 12, c * 1536 + t * 128 + p,
        np.where(t < 24, 12288 + c * 1536 + (t - 12) * 128 + p,
                 DENSE_ROWS + c * 3200 + (t - 24) * 128 + p))


def _preprocess(x, edge_index, W, b):
    fp8 = ml_dtypes.float8_e4m3
    src_old = edge_index[0].astype(np.int64)
    dst0 = edge_index[1].astype(np.int64)
    loops = np.arange(N, dtype=np.int64)
    deg = np.bincount(np.concatenate([dst0, loops]), minlength=N).astype(np.float32)
    dis = np.zeros(NPAD, np.float32)
    dis[:N] = 1.0 / np.sqrt(deg)

    perm = _node_perm()
    src0 = perm[src_old]
    core_of = dst0 // NPC
    is_dense = src0 < DENSE_ROWS

    # ---------------- dense path: B count blocks, group-major ----------------
    # BG[c] layout: [NGROUPS, NSB, 128(u), SBW(k), GW(dst col)]
    BG = np.zeros((NCORES, NGROUPS, NSB, 128, SBW, GW), np.uint8)
    de = np.where(is_dense)[0]
    d_core = core_of[de]
    st = src0[de] // 128
    d_sb = st // SBW
    d_k = st % SBW
    d_u = src0[de] % 128
    d_dloc = dst0[de] - d_core * NPC
    d_g = d_dloc // GW
    d_col = d_dloc % GW
    np.add.at(BG, (d_core, d_g, d_sb, d_u, d_k, d_col), 1)

    # ---------------- gather path ----------------
    ge = np.where(~is_dense)[0]
    gsrc, gdst = src0[ge], dst0[ge]
    g_core = core_of[ge]
    lt_of = (gdst % NPC) // 128
    dloc_of = gdst % 128
    key = g_core * TPC + lt_of
    order = np.argsort(key, kind="stable")
    src_s = gsrc[order]
    dloc_s = dloc_of[order]
    counts = np.bincount(key, minlength=NCORES * TPC).reshape(NCORES, TPC)
    KCH = np.maximum(np.ceil(counts.max(axis=0) / 128.0).astype(np.int64), 1)  # [TPC]

    tiles_of_group = [list(range(g * GROUP, min((g + 1) * GROUP, TPC)))
                      for g in range(NGROUPS)]
    TOTCH = int(KCH.sum())
    chunk_off = np.zeros(TPC, np.int64)
    pos = 0
    for tg in tiles_of_group:
        for t in tg:
            chunk_off[t] = pos
            pos += KCH[t]
    assert pos == TOTCH

    idx_all = np.zeros((NCORES, TOTCH * 128), np.int16)
    dlocs = np.full((NCORES, TOTCH * 128), -1, np.int16)
    starts = np.zeros(NCORES * TPC + 1, np.int64)
    np.cumsum(counts.reshape(-1), out=starts[1:])
    for c in range(NCORES):
        for t in range(TPC):
            k = c * TPC + t
            s0, s1 = starts[k], starts[k + 1]
            n = s1 - s0
            o = chunk_off[t] * 128
            idx_all[c, o:o + n] = (src_s[s0:s1] - DENSE_ROWS).astype(np.int16)
            dlocs[c, o:o + n] = dloc_s[s0:s1]

    smat = np.zeros((NCORES, TOTCH, 128, 128), fp8)
    ii = np.arange(TOTCH * 128)
    for c in range(NCORES):
        d = dlocs[c]
        m = d >= 0
        smat[c][ii[m] // 128, ii[m] % 128, d[m]] = 1.0

    idxw = np.zeros((NCORES, 128, TOTCH * 8), np.int16)
    for c in range(NCORES):
        w = idx_all[c].reshape(TOTCH * 8, 16).T
        for gme in range(8):
            idxw[c, gme * 16:(gme + 1) * 16, :] = w

    xpad = np.zeros((NPAD, D), np.float32)
    xpad[:N] = x
    x0p_node = (xpad * dis[:, None]).astype(ml_dtypes.bfloat16)
    x0p = np.empty_like(x0p_node)
    x0p[perm] = x0p_node  # table in rank-major-by-half layout
    disT = dis.reshape(NCORES, TPC, 128).transpose(0, 2, 1).copy()  # [c,128,TPC]
    b_bc = np.broadcast_to(b[None, :, :], (128, L, D)).astype(np.float32).copy()
    ident = np.eye(128, dtype=np.float32).astype(ml_dtypes.bfloat16)

    meta = dict(KCH=KCH, TOTCH=TOTCH, chunk_off=chunk_off,
                tiles_of_group=tiles_of_group)
    per_core = []
    for c in range(NCORES):
        x0p_own = np.ascontiguousarray(
            x0p_node[c * NPC:(c + 1) * NPC].reshape(TPC, 128, D).transpose(1, 0, 2))
        per_core.append(dict(
            x_own=np.ascontiguousarray(xpad[c * NPC:(c + 1) * NPC]),
            x0p=x0p,
            x0p_own=x0p_own,
            w=W.astype(ml_dtypes.bfloat16),
            b_bc=b_bc,
            disT=np.ascontiguousarray(disT[c]),
            ident=ident,
            idxs=np.ascontiguousarray(idxw[c]),
            smat=np.ascontiguousarray(smat[c].transpose(1, 0, 2)),
            bg=BG[c].astype(fp8),
        ))
    return meta, per_core


def _build(meta):
    from concourse import bacc, tile
    from concourse.bass import mybir

    KCH = meta["KCH"]
    TOTCH = meta["TOTCH"]
    chunk_off = meta["chunk_off"]
    tiles_of_group = meta["tiles_of_group"]
    MAXCH_U = max(int(sum(KCH[t] for t in u))
                  for tg in tiles_of_group for u in _units_of_group(tg))

    nc = bacc.Bacc("TRN2", target_bir_lowering=False, debug=False,
                   num_devices=NCORES)
    d_x_own = nc.dram_tensor("x_own", [NPC, D], mybir.dt.float32, kind="ExternalInput")
    d_x0p = nc.dram_tensor("x0p", [NPAD, D], mybir.dt.bfloat16, kind="ExternalInput")
    d_x0po = nc.dram_tensor("x0p_own", [128, TPC, D], mybir.dt.bfloat16, kind="ExternalInput")
    d_w = nc.dram_tensor("w", [L, D, D], mybir.dt.bfloat16, kind="ExternalInput")
    d_bbc = nc.dram_tensor("b_bc", [128, L, D], mybir.dt.float32, kind="ExternalInput")
    d_disT = nc.dram_tensor("disT", [128, TPC], mybir.dt.float32, kind="ExternalInput")
    d_ident = nc.dram_tensor("ident", [128, 128], mybir.dt.bfloat16, kind="ExternalInput")
    d_idxs = nc.dram_tensor("idxs", [128, TOTCH * 8], mybir.dt.int16, kind="ExternalInput")
    d_smat = nc.dram_tensor("smat", [128, TOTCH, 128], mybir.dt.float8e4, kind="ExternalInput")
    d_bg = nc.dram_tensor("bg", [NGROUPS, NSB, 128, SBW, GW], mybir.dt.float8e4,
                          kind="ExternalInput")
    d_out = nc.dram_tensor("x_out", [NPC, D], mybir.dt.float32, kind="ExternalOutput")

    with tile.TileContext(nc) as tc:
        with (
            tc.tile_pool(name="const", bufs=1) as constp,
            tc.tile_pool(name="xallp", bufs=1) as xallp,
            tc.tile_pool(name="msg", bufs=4) as msgp,
            tc.tile_pool(name="sp", bufs=4) as sp_pool,
            tc.tile_pool(name="bblk", bufs=2) as bbp,
            tc.tile_pool(name="work", bufs=3) as workp,
            tc.tile_pool(name="aggt", bufs=2) as aggp,
            tc.tile_pool(name="pag", bufs=4, space="PSUM") as pagp,
            tc.tile_pool(name="pout", bufs=2, space="PSUM") as poutp,
            tc.tile_pool(name="dram", bufs=1, space="DRAM") as dramp,
        ):
            x_own = constp.tile([128, TPC, D], mybir.dt.float32, tag="x_own")
            nc.sync.dma_start(out=x_own[:], in_=d_x_own.ap().rearrange("(t p) f -> p t f", p=128))
            xpr_all = constp.tile([128, TPC, D], mybir.dt.bfloat16, tag="xpr_all")
            nc.sync.dma_start(out=xpr_all[:], in_=d_x0po.ap())
            w_sb = constp.tile([128, L, D], mybir.dt.bfloat16, tag="w_sb")
            nc.sync.dma_start(out=w_sb[:], in_=d_w.ap().rearrange("l k f -> k l f"))
            bbc_sb = constp.tile([128, L, D], mybir.dt.float32, tag="bbc")
            nc.sync.dma_start(out=bbc_sb[:], in_=d_bbc.ap())
            disT_sb = constp.tile([128, TPC], mybir.dt.float32, tag="disT")
            nc.sync.dma_start(out=disT_sb[:], in_=d_disT.ap())
            ident_sb = constp.tile([128, 128], mybir.dt.bfloat16, tag="ident")
            nc.sync.dma_start(out=ident_sb[:], in_=d_ident.ap())
            idx_sb = constp.tile([128, TOTCH * 8], mybir.dt.int16, tag="idx")
            nc.sync.dma_start(out=idx_sb[:], in_=d_idxs.ap())

            tables = [d_x0p.ap()]
            ag_ins_a1 = []
            ag_ins_a2 = []
            ag_ins_b = []
            for l in range(1, L):
                tab_tile = dramp.tile([NPAD, D], mybir.dt.bfloat16, tag=f"tab{l}")
                agina1_tile = dramp.tile([12 * 128, D], mybir.dt.bfloat16, tag=f"agina1{l}")
                agina2_tile = dramp.tile([12 * 128, D], mybir.dt.bfloat16, tag=f"agina2{l}")
                aginb_tile = dramp.tile([25 * 128, D], mybir.dt.bfloat16, tag=f"aginb{l}")
                tables.append(tab_tile[:])
                ag_ins_a1.append(agina1_tile[:])
                ag_ins_a2.append(agina2_tile[:])
                ag_ins_b.append(aginb_tile[:])

            GROUP_ORDER = list(range(6, NGROUPS)) + list(range(0, 6))
            for l in range(L):
                table = tables[l]
                tabG = table[DENSE_ROWS:NPAD, :]
                xall = xallp.tile([128, M_DENSE, D], mybir.dt.bfloat16,
                                  tag="xall", name=f"xall_{l}")
                nc.sync.dma_start(
                    out=xall[:, 0:96, :],
                    in_=table[0:12288, :].rearrange("(t p) f -> p t f", p=128))
                nc.sync.dma_start(
                    out=xall[:, 96:192, :],
                    in_=table[12288:DENSE_ROWS, :].rearrange("(t p) f -> p t f", p=128))
                for g in GROUP_ORDER:
                    tg = tiles_of_group[g]
                    psum_g = pagp.tile([128, GW], mybir.dt.float32, tag="pag",
                                       name=f"pag_{l}_{g}")
                    units = []
                    for u in _units_of_group(tg):
                        nch = int(sum(KCH[t] for t in u))
                        goff = int(chunk_off[u[0]])
                        s_t = sp_pool.tile([128, MAXCH_U, 128], mybir.dt.float8e4,
                                           tag="s_t", name=f"s_t_{l}_{g}_{u[0]}")
                        nc.sync.dma_start(
                            out=s_t[:, 0:nch, :],
                            in_=d_smat.ap()[:, goff:goff + nch, :])
                        msg = msgp.tile([128, MAXCH_U, D], mybir.dt.bfloat16,
                                        tag="msg", name=f"msg_{l}_{g}_{u[0]}")
                        nc.gpsimd.dma_gather(
                            out_ap=msg[:, 0:nch, :],
                            in_ap=tabG,
                            idxs_ap=idx_sb[:, goff * 8:(goff + nch) * 8],
                            num_idxs=nch * 128,
                            num_idxs_reg=nch * 128,
                            elem_size=D,
                            single_packet=False,
                        )
                        units.append((u, goff, s_t, msg))
                    for sb in range(NSB):
                        bsub = bbp.tile([128, SBW, GW], mybir.dt.float8e4, tag="bsub",
                                        name=f"bsub_{l}_{g}_{sb}")
                        nc.sync.dma_start(out=bsub[:], in_=d_bg.ap()[g, sb])
                        for k in range(SBW):
                            nc.tensor.matmul(
                                psum_g[:],
                                lhsT=xall[:, sb * SBW + k, :],
                                rhs=bsub[:, k, :],
                                start=(sb == 0 and k == 0), stop=False)
                    for u, goff, s_t, msg in units:
                        for t in u:
                            tl = t - tg[0]
                            oT = int(chunk_off[t] - goff)
                            for c in range(int(KCH[t])):
                                nc.tensor.matmul(
                                    psum_g[:, tl * 128:(tl + 1) * 128],
                                    lhsT=msg[:, oT + c, :],
                                    rhs=s_t[:, oT + c, :],
                                    start=False, stop=False)
                    for tl, t in enumerate(tg):
                        nc.tensor.matmul(
                            psum_g[:, tl * 128:(tl + 1) * 128],
                            lhsT=xpr_all[:, t, :],
                            rhs=ident_sb[:],
                            start=False, stop=(tl == len(tg) - 1))
                    wid = len(tg) * 128
                    aggT = aggp.tile([128, GW], mybir.dt.bfloat16, tag="aggT",
                                     name=f"aggT_{l}_{g}")
                    nc.scalar.copy(aggT[:, 0:wid], psum_g[:, 0:wid])
                    for tl, t in enumerate(tg):
                        pso = poutp.tile([128, 128], mybir.dt.float32, tag="pout",
                                         name=f"pout_{l}_{t}")
                        nc.tensor.matmul(pso[:], lhsT=aggT[:, tl * 128:(tl + 1) * 128],
                                         rhs=w_sb[:, l, :], start=True, stop=True)
                        prevb = workp.tile([128, 128], mybir.dt.float32, tag="prevb")
                        nc.vector.tensor_tensor(
                            out=prevb[:], in0=x_own[:, t, :], in1=bbc_sb[:, l, :],
                            op=mybir.AluOpType.add)
                        t2 = workp.tile([128, 128], mybir.dt.float32, tag="t2")
                        nc.vector.tensor_scalar(
                            out=t2[:], in0=pso[:], scalar1=disT_sb[:, t:t + 1],
                            scalar2=None, op0=mybir.AluOpType.mult)
                        t3 = workp.tile([128, 128], mybir.dt.float32, tag="t3")
                        nc.vector.tensor_tensor(
                            out=t3[:], in0=t2[:], in1=prevb[:], op=mybir.AluOpType.add)
                        nc.scalar.activation(
                            out=x_own[:, t, :], in_=t3[:],
                            func=mybir.ActivationFunctionType.Relu)
                        if l < L - 1:
                            nc.scalar.activation(
                                out=xpr_all[:, t, :], in_=x_own[:, t, :],
                                func=mybir.ActivationFunctionType.Copy,
                                scale=disT_sb[:, t:t + 1])
                    if l < L - 1:
                        t0, t1 = tg[0], tg[-1] + 1
                        if t0 >= 24:
                            nc.sync.dma_start(
                                out=ag_ins_b[l].rearrange("(t p) f -> p t f", p=128)
                                    [:, t0 - 24:t1 - 24, :],
                                in_=xpr_all[:, t0:t1, :])
                        elif t0 >= 12:
                            nc.sync.dma_start(
                                out=ag_ins_a2[l].rearrange("(t p) f -> p t f", p=128)
                                    [:, t0 - 12:t1 - 12, :],
                                in_=xpr_all[:, t0:t1, :])
                        else:
                            nc.sync.dma_start(
                                out=ag_ins_a1[l].rearrange("(t p) f -> p t f", p=128)
                                    [:, t0:t1, :],
                                in_=xpr_all[:, t0:t1, :])
                    # progressive AllGathers: B after the part-B groups
                    # (processed first) so next layer's gathers never wait;
                    # A1 after groups 0-2 so the dense xall first half is
                    # ready before the next layer starts; A2 at layer end.
                    if l < L - 1 and g == NGROUPS - 1:
                        nc.gpsimd.collective_compute(
                            "AllGather",
                            mybir.AluOpType.bypass,
                            replica_groups=[list(range(NCORES))],
                            ins=[ag_ins_b[l].opt()],
                            outs=[tables[l + 1][DENSE_ROWS:NPAD, :].opt()],
                        )
                    if l < L - 1 and g == 2:
                        nc.gpsimd.collective_compute(
                            "AllGather",
                            mybir.AluOpType.bypass,
                            replica_groups=[list(range(NCORES))],
                            ins=[ag_ins_a1[l].opt()],
                            outs=[tables[l + 1][0:12288, :].opt()],
                        )
                    if l < L - 1 and g == 5:
                        nc.gpsimd.collective_compute(
                            "AllGather",
                            mybir.AluOpType.bypass,
                            replica_groups=[list(range(NCORES))],
                            ins=[ag_ins_a2[l].opt()],
                            outs=[tables[l + 1][12288:DENSE_ROWS, :].opt()],
                        )
            nc.sync.dma_start(out=d_out.ap().rearrange("(t p) f -> p t f", p=128),
                              in_=x_own[:])

    nc.compile()
    return nc


def kernel(x, edge_index, W, b):
    global _compiled
    from concourse import bass_utils

    x = np.asarray(x, dtype=np.float32)
    W_np = np.asarray(W, dtype=np.float32)
    b_np = np.asarray(b, dtype=np.float32)
    ei = np.asarray(edge_index)

    meta, per_core = _preprocess(x, ei, W_np, b_np)
    globals()["_last_per_core"] = per_core
    if _compiled is None:
        _compiled = _build(meta)
    nc = _compiled
    res = bass_utils.run_bass_kernel_spmd(nc, per_core, core_ids=list(range(NCORES)))
    out = np.concatenate([res.results[c]["x_out"] for c in range(NCORES)], axis=0)
    return out[:N].astype(np.float32)
